# revision 3
# baseline (speedup 1.0000x reference)
"""Trainium2 Bass kernel for a 2-layer LSTM encoder (relu cell activation), v10: v5 with LAG reduced 32->16 (TOT 544->528); L2 bulk slices write directly into the consumer-step zx rows and are emitted immediately so they stay 7+ steps ahead of their consumers.

Problem: x[128, 512, 64] -> LSTM(256, relu, seq) -> LSTM(128, relu, last) -> out[128, 128]

v2 over the v1 design: the per-step critical cycle is latency-bound on the
cross-engine elementwise chain (PE matmuls -> ACT sigmoid -> 4 serial DVE
ops -> PE). v2 cuts the chain to THREE DVE instructions and removes ACT
entirely from the cycle, using a fused custom DVE op:

    LSTM_SIGMUL5:  out = (z'*((z'^2 - A)^2 + B) + 0.5) * relu(Src1)

where z' = gamma*z is the gate preactivation with the i/f/o weight blocks
pre-scaled by gamma host-side, so that the degree-5 odd minimax polynomial
for sigmoid on [-3.2, 3.2] (preactivations measured in [-2.7, 2.7]) has
leading coefficient 1 and fits the 3 scalar slots (A, B, 0.5) of the custom
DVE encoding.  One instance computes [i*relu(g) | f*c] (96 cols, both
layers), a stock add produces c', and a second instance computes
h = sigmoid(o)*relu(c') (note c >= 0 always, so relu(c)=c exactly; relu in
the op is needed only for the g half of instance 1 and is harmless on c).

PSUM has a single DVE read port, so the custom op can take at most one PSUM
operand: in0 = z'_if stays in PSUM, and Src1 = [g | c] lives in SBUF
(gc_sb), with one DVE copy moving the raw g block PSUM->SBUF each step (the
relu lives inside the fused op, so it is a plain copy).
"""

import numpy as np
from contextlib import ExitStack

import concourse.bass as bass
import concourse.tile as tile
from concourse import bacc
from concourse import mybir
from concourse.bass_utils import run_bass_kernel_spmd
from concourse.dve_spec import Spec, Src0, Src1, C0, C1, C2, relu as dve_relu, lower
import concourse.dve_ops as dve_ops
from concourse.dve_ops import DveOp
from concourse.dve_table_gen import dve_ver_for
from concourse.dve_uop import DveOpSpec

fp32 = mybir.dt.float32
bf16 = mybir.dt.bfloat16
AF = mybir.ActivationFunctionType

B, T, F = 128, 512, 64
U1, U2 = 256, 128
NCORES = 8
BL = B // NCORES  # 16 batch rows per core

# deg-5 odd minimax fit of sigmoid(x)-0.5 on [-3.2, 3.2]:
#   sigmoid(x) ~ 0.5 + c5*x*((x^2-a)^2 + b2)
# with z' = GAMMA*x (GAMMA = c5^(1/5)) this is 0.5 + z'*((z'^2-SA)^2+SB).
_C1, _C3, _C5 = 0.2448235684267281, -0.015655168381943876, 0.0005737185431172905
_a = -(_C3 / _C5) / 2.0
_b2 = (_C1 / _C5) - _a * _a
GAMMA = _C5 ** 0.2
SA = GAMMA * GAMMA * _a
SB = GAMMA ** 4 * _b2

# gate block order in the z tile: I, F, O, G; Keras weight column order is
# i, f, g, o -> column offsets per block:
COLMAP1 = [0 * U1, 1 * U1, 3 * U1, 2 * U1]  # into [*, 4*U1]
COLMAP2 = [0 * U2, 1 * U2, 3 * U2, 2 * U2]  # into [*, 4*U2]

# packed constant blocks (single DMA each to limit sync-wait fan-in)
CB16_COLS = 8 * U1 + 12 * U2 + 128   # u1(2x1024) | w2(2x512) | u2(512) | eye(128)
CF32_COLS = 4 * U1 + 128 + 8 + 4     # w1(1024, rows0:64) | eye(128) | b1p(8) | b2p(4)

# step psum tile regions: [IF(0:96) | O(96:144) | G(144:192) | C(192:240)]
RIF, RO, RG, RC = 0, 96, 144, 192


def _register_sigmul():
    ver = dve_ver_for("TRN2")
    z = Src0
    u = z * z
    d = u - C0
    m = d * d + C1
    s = z * m + C2
    spec = Spec(body=s * dve_relu(Src1))
    if "LSTM_SIGMUL5" in dve_ops._SUB_OPCODE_FOR_NAME:
        for op in dve_ops.OPS:
            if op.name == "LSTM_SIGMUL5":
                return op
    uops = lower(spec, ver=ver)
    opcode = max(dve_ops._SUB_OPCODE_FOR_NAME.values()) + 1
    sha = DveOpSpec(name="LSTM_SIGMUL5", opcode=opcode, uops=uops,
                    rd1_en=True).sha(ver)
    op = DveOp("LSTM_SIGMUL5", spec, subdim=False, uops_sha={ver: sha})
    dve_ops.OPS.append(op)
    dve_ops.CUSTOM_DVE_SPECS[op.name] = op.spec
    dve_ops._SUB_OPCODE_FOR_NAME[op.name] = opcode
    return op


SIGMUL = _register_sigmul()


def build(T_=T, CH=32, nonzero_bias=False, reps=1):
    """Build the per-core Bass program. Returns nc.

    reps>1 repeats the whole computation (for timing-by-differencing)."""
    assert T_ % CH == 0 and (CH * BL) % 128 == 0
    NCH = T_ // CH
    NJ = CH * BL // 128  # 128-row blocks per x chunk
    RING = 3 * CH  # h ring slots (multiple of CH, covers lag + slack)
    LAG = 16       # L2 lags L1 by two sub-slices
    TOT = T_ + LAG

    nc = bacc.Bacc("TRN2", target_bir_lowering=False, debug=False)

    x_d = nc.declare_dram_parameter("x", [T_ * BL, F], fp32, isOutput=False)
    cb_d = nc.declare_dram_parameter("cb16", [128, CB16_COLS], bf16, isOutput=False)
    cf_d = nc.declare_dram_parameter("cf32", [128, CF32_COLS], fp32, isOutput=False)
    out_d = nc.declare_dram_parameter("out", [BL, U2], fp32, isOutput=True)

    with tile.TileContext(nc) as tc, ExitStack() as ctx:
        const_p = ctx.enter_context(tc.tile_pool(name="const", bufs=1))
        xst_p = ctx.enter_context(tc.tile_pool(name="xst", bufs=2))
        xT_p = ctx.enter_context(tc.tile_pool(name="xT", bufs=2))
        zx_p = ctx.enter_context(tc.tile_pool(name="zx", bufs=3))
        ew_p = ctx.enter_context(tc.tile_pool(name="ew", bufs=3))
        state_p = ctx.enter_context(tc.tile_pool(name="state", bufs=1))
        zt_p = ctx.enter_context(tc.tile_pool(name="zt", bufs=1, space="PSUM"))
        pb1_p = ctx.enter_context(tc.tile_pool(name="pb1", bufs=2, space="PSUM"))
        pb2_p = ctx.enter_context(tc.tile_pool(name="pb2", bufs=2, space="PSUM"))

        # ---- load all constants with TWO DMAs (avoids sync-wait fan-in) ----
        cb = const_p.tile([128, CB16_COLS], bf16, name="cb")
        nc.sync.dma_start(cb[:, :], cb_d[:, :])
        cf = const_p.tile([128, CF32_COLS], fp32, name="cf")
        nc.sync.dma_start(cf[:, :], cf_d[:, :])
        # bf16 views
        u1sb = [cb[:, 0:4 * U1], cb[:, 4 * U1:8 * U1]]
        w2sb = [cb[:, 8 * U1:8 * U1 + 4 * U2],
                cb[:, 8 * U1 + 4 * U2:8 * U1 + 8 * U2]]
        u2sb = cb[0:U2, 8 * U1 + 8 * U2:8 * U1 + 12 * U2]
        idb = cb[:, 8 * U1 + 12 * U2:8 * U1 + 12 * U2 + 128]
        # fp32 views
        w1sb = cf[0:F, 0:4 * U1]
        idf = cf[:, 4 * U1:4 * U1 + 128]
        b1sb = cf[:, 4 * U1 + 128:4 * U1 + 136]
        b2sb = cf[:, 4 * U1 + 136:4 * U1 + 140]

        # ---- persistent state ----
        # gc: [128, (g_raw | c)] x 48 in SBUF; the custom op reads [g|c] as
        # one stream (relu applied in-op; harmless on c since c >= 0)
        gc_sb = state_p.tile([128, 96], fp32)
        c_sb = gc_sb[:, 48:96]
        # h ring: slot t%RING -> [128, (h1_uc0|h1_uc1|h2), 16b] bf16
        h_ring = state_p.tile([128, RING, 48], bf16)
        # step gate tiles, even/odd parity; padded to a full psum bank each so
        # PE writes of step t+1 and DVE reads of step t hit different banks.
        ZT = [zt_p.tile([128, 512], fp32, name=f"zt{i}") for i in range(2)]

        # fence the preamble (const DMA) so later instructions sync through
        # one barrier instead of fanning in on many queues
        tc.strict_bb_all_engine_barrier()

        # per-chunk zx buffers: chunk k tile holds L1 x-part for chunk k and
        # L2 x-part (h1@W2) for chunk k-1, layout [I(48)|F(48)|O(48)|G(48)],
        # each block [uc0|uc1|L2] x 16b.
        zx1_tiles = [None] * (NCH + 1)

        def _get_zx(k):
            if zx1_tiles[k] is None:
                zx1 = zx_p.tile([128, CH, 192], bf16, name="zx1", tag="zx1")
                zx1_tiles[k] = zx1
                if k == 0 or k >= NCH:
                    # unwritten columns are injected before being overwritten;
                    # clear once so no stray NaN bit patterns enter PSUM
                    nc.vector.memset(zx1[:, :, :], 0.0)
            return zx1_tiles[k]

        MSUB = 2
        MT = CH // MSUB
        LSUB = 4
        LT = CH // LSUB

        def bulk_l1x_ops(k):
            holder = {}

            def _dma():
                xst = xst_p.tile([128, NJ, F], fp32, name="xst")
                holder["xst"] = xst
                nc.sync.dma_start(
                    xst[:, :, :],
                    x_d.rearrange("(c j p) f -> c p j f", j=NJ, p=128)[k],
                )
                holder["xTc"] = xT_p.tile([F, CH * BL], fp32, name="xTc")
            yield _dma

            def _tr(j):
                def _f():
                    ptx = pb1_p.tile([F, 128], fp32, name="ptx", tag="pb1")
                    nc.tensor.transpose(ptx[:, :], holder["xst"][:, j, :], idf[:, :])
                    nc.scalar.copy(holder["xTc"][:, j * 128:(j + 1) * 128], ptx[:, :])
                return _f
            for j in range(NJ):
                yield _tr(j)

            zx1 = _get_zx(k)

            def _mm(bi, uc, ms):
                def _f():
                    pb = pb1_p.tile([128, MT * BL], fp32, name="pb", tag="pb1")
                    nc.tensor.matmul(
                        pb[:, :],
                        w1sb[:, COLMAP1[bi] + uc * 128:COLMAP1[bi] + (uc + 1) * 128],
                        holder["xTc"][:, ms * MT * BL:(ms + 1) * MT * BL],
                        start=True, stop=True,
                    )
                    src = pb.rearrange("p (t b) -> p t b", b=BL)
                    dst = zx1[:, ms * MT:(ms + 1) * MT,
                              bi * 48 + uc * 16:bi * 48 + (uc + 1) * 16]
                    if nonzero_bias:
                        nc.vector.tensor_scalar_add(
                            dst, src, b1sb[:, bi * 2 + uc:bi * 2 + uc + 1])
                    else:
                        nc.scalar.copy(dst, src)
                return _f
            for bi in range(4):
                for uc in range(2):
                    for ms in range(MSUB):
                        yield _mm(bi, uc, ms)

        def bulk_l2x_ops(k, sj):
            tcons = k * CH + sj * LT + LAG      # first consumer step
            zx2 = _get_zx(tcons // CH)
            row = tcons % CH
            rs = (k * CH + sj * LT) % RING

            def _one(bi):
                def _f():
                    pb = pb2_p.tile([128, LT * BL], fp32, name="pb2t", tag="pb2")
                    for kc in range(2):
                        nc.tensor.matmul(
                            pb[:, :],
                            w2sb[kc][:, COLMAP2[bi]:COLMAP2[bi] + 128],
                            h_ring[:, rs:rs + LT, kc * 16:(kc + 1) * 16],
                            start=(kc == 0), stop=(kc == 1),
                        )
                    src = pb.rearrange("p (t b) -> p t b", b=BL)
                    dst = zx2[:, row:row + LT, bi * 48 + 32:bi * 48 + 48]
                    if nonzero_bias:
                        nc.vector.tensor_scalar_add(dst, src, b2sb[:, bi:bi + 1])
                    else:
                        nc.scalar.copy(dst, src)
                return _f
            for bi in range(4):
                yield _one(bi)

        def emit_body():
            nonlocal h2f
            zx1_tiles[:] = [None] * (NCH + 1)
            for op in bulk_l1x_ops(0):
                op()
            pending = []
            for t in range(TOT):
                k, tl = t // CH, t % CH
                s = t - LAG  # layer-2 step
                if t < T_ and tl == 0 and k + 1 < NCH:
                    pending.extend(bulk_l1x_ops(k + 1))

                zt = ZT[t % 2]
                zxt = zx1_tiles[k]
                hp = h_ring[:, (t - 1) % RING, :]

                # matmul sequences: G first (feeds SIGMUL#1 in1), then IF
                # (feeds SIGMUL#1 in0), then O (only needed one DVE op later).
                def useq(bi, dst_lo, width_inject):
                    seq = [(zt[:, dst_lo:dst_lo + width_inject], idb[:, :],
                            zxt[:, tl, dst_lo:dst_lo + width_inject])]
                    bis = [bi] if isinstance(bi, int) else bi
                    for b in bis:
                        off = dst_lo if isinstance(bi, int) else dst_lo + (b - bis[0]) * 48
                        if t < T_:
                            for uc in range(2):
                                cc = COLMAP1[b] + uc * 128
                                for kc in range(2):
                                    seq.append((zt[:, off + uc * 16:off + (uc + 1) * 16],
                                                u1sb[kc][:, cc:cc + 128],
                                                hp[:, kc * 16:(kc + 1) * 16]))
                        if s >= 0:
                            seq.append((zt[:, off + 32:off + 48],
                                        u2sb[:, COLMAP2[b]:COLMAP2[b] + 128],
                                        hp[:, 32:48]))
                    for i, (o, l, r) in enumerate(seq):
                        nc.tensor.matmul(o, l, r, start=(i == 0),
                                         stop=(i == len(seq) - 1))

                useq(3, RG, 48)        # G first (feeds the g-copy early)
                useq(2, RO, 48)        # O second (covered by SIG1's IF wait)
                useq([0, 1], RIF, 96)  # I,F last

                # 4-op cell update, all DVE (ACT not on the cycle at all):
                #   g_sb = copy(z_g)                  (PSUM -> SBUF, no relu)
                #   igfc = sig(z_if) * relu([g | c])
                #   c'   = igfc[0:48] + igfc[48:96]
                #   h    = sig(z_o) * relu(c')
                nc.vector.tensor_copy(gc_sb[:, 0:48], zt[:, RG:RG + 48])
                igfc = ew_p.tile([128, 96], fp32, name="igfc")
                nc.vector._custom_dve(
                    SIGMUL, out=igfc[:, :], in0=zt[:, RIF:RIF + 96],
                    in1=gc_sb[:, :], s0=SA, s1=SB, imm2=0.5)
                nc.vector.tensor_add(c_sb, igfc[:, 0:48], igfc[:, 48:96])
                slot = t % RING
                nc.vector._custom_dve(
                    SIGMUL, out=h_ring[:, slot, :], in0=zt[:, RO:RO + 48],
                    in1=c_sb, s0=SA, s1=SB, imm2=0.5)

                if t == LAG - 1:
                    # reset L2 state before its first real step
                    nc.vector.memset(h_ring[:, slot, 32:48], 0.0)
                    nc.vector.memset(gc_sb[:, 80:96], 0.0)
                if t == TOT - 1:
                    h2f = ew_p.tile([128, 48], fp32, name="h2f")
                    nc.vector._custom_dve(
                        SIGMUL, out=h2f[:, :], in0=zt[:, RO:RO + 48],
                        in1=c_sb, s0=SA, s1=SB, imm2=0.5)

                # L2 bulk slices: emitted AFTER this step's SIG2 (the last
                # slice of a chunk reads h written this very step)
                if t < T_ and tl % LT == LT - 1 and k < NCH:
                    for op in bulk_l2x_ops(k, tl // LT):
                        op()

                budget = 2 if pending and len(pending) > (CH - tl) else 1
                for _ in range(budget):
                    if pending:
                        pending.pop(0)()

            while pending:
                pending.pop(0)()

        h2f = None
        for _rep in range(reps):
            nc.vector.memset(gc_sb[:, :], 0.0)
            nc.vector.memset(h_ring[:, RING - 1, :], 0.0)  # h(-1) = 0
            emit_body()

        # final: h2 [128u, 16b] -> out [16b, 128u]
        pfin = pb1_p.tile([BL, 128], fp32, name="pfin", tag="pb1")
        nc.tensor.transpose(pfin[:, :], h2f[:, 32:48], idf[:, :])
        osb = ew_p.tile([BL, 128], fp32, name="osb")
        nc.scalar.copy(osb[:, :], pfin[:, :])
        nc.sync.dma_start(out_d[:, :], osb[:, :])

    nc.finalize()
    return nc


_cache = {}


def _get_nc(T_=T, CH=32, nonzero_bias=False, reps=1):
    key = (T_, CH, nonzero_bias, reps)
    if key not in _cache:
        _cache[key] = build(T_, CH, nonzero_bias, reps)
    return _cache[key]


def make_inputs(x, W1, U1w, b1, W2, U2w, b2, T_=T):
    """Host-side packing -> per-core input maps.

    The i/f/o gate blocks of every weight/bias are pre-scaled by GAMMA so the
    kernel's PSUM preactivations arrive as z' = GAMMA*z for the sigmoid
    polynomial; the g block is unscaled."""
    bf = mybir.dt.np(bf16)
    g32 = np.float32(GAMMA)
    sc1 = np.ones(4 * U1, np.float32)
    sc1[0:2 * U1] = g32          # i, f
    sc1[3 * U1:4 * U1] = g32     # o
    sc2 = np.ones(4 * U2, np.float32)
    sc2[0:2 * U2] = g32
    sc2[3 * U2:4 * U2] = g32
    x = np.asarray(x, np.float32)
    W1s = np.asarray(W1, np.float32) * sc1
    U1s = np.asarray(U1w, np.float32) * sc1
    W2s = np.asarray(W2, np.float32) * sc2
    U2s = np.asarray(U2w, np.float32) * sc2
    b1s = np.asarray(b1, np.float32) * sc1
    b2s = np.asarray(b2, np.float32) * sc2
    u1b = U1s.astype(bf)
    u2b = U2s.astype(bf)
    w2b = W2s.astype(bf)
    b1p = np.zeros((128, 8), np.float32)
    for bi in range(4):
        for uc in range(2):
            b1p[:, bi * 2 + uc] = b1s[COLMAP1[bi] + uc * 128:COLMAP1[bi] + (uc + 1) * 128]
    b2p = np.zeros((128, 4), np.float32)
    for bi in range(4):
        b2p[:, bi] = b2s[COLMAP2[bi]:COLMAP2[bi] + 128]
    cb16 = np.zeros((128, CB16_COLS), bf)
    cb16[:, 0:1024] = u1b[0:128]
    cb16[:, 1024:2048] = u1b[128:256]
    cb16[:, 2048:2560] = w2b[0:128]
    cb16[:, 2560:3072] = w2b[128:256]
    cb16[:, 3072:3584] = u2b
    cb16[:, 3584:3712] = np.eye(128).astype(bf)
    cf32 = np.zeros((128, CF32_COLS), np.float32)
    cf32[0:64, 0:1024] = W1s
    cf32[:, 1024:1152] = np.eye(128, dtype=np.float32)
    cf32[:, 1152:1160] = b1p
    cf32[:, 1160:1164] = b2p
    common = dict(cb16=cb16, cf32=cf32)
    xr = x.reshape(NCORES, BL, x.shape[1], F)
    in_maps = []
    for c in range(NCORES):
        xc = np.ascontiguousarray(
            np.swapaxes(xr[c], 0, 1).reshape(x.shape[1] * BL, F))
        m = dict(common)
        m["x"] = xc[:T_ * BL]
        in_maps.append(m)
    nonzero_bias = bool(np.any(b1) or np.any(b2))
    return in_maps, nonzero_bias


def run(inputs, T_=T, CH=32, trace=False, reps=1):
    """inputs: dict from reference.setup_inputs(). Returns (out, exec_time_ns)."""
    in_maps, nzb = make_inputs(
        inputs["x"], inputs["W1"], inputs["U1"], inputs["b1"],
        inputs["W2"], inputs["U2"], inputs["b2"], T_=T_)
    nc = _get_nc(T_, CH, nzb, reps)
    res = run_bass_kernel_spmd(nc, in_maps, list(range(NCORES)), trace=trace)
    out = np.concatenate(
        [res.results[c]["out"] for c in range(NCORES)], axis=0)
    return np.ascontiguousarray(out, dtype=np.float32), res.exec_time_ns


def kernel(x, W1, U1, b1, W2, U2, b2):
    out, _ = run(dict(x=x, W1=W1, U1=U1, b1=b1, W2=W2, U2=U2, b2=b2))
    return out
